# revision 17
# baseline (speedup 1.0000x reference)
"""Trainium2 Bass kernel for nn_ConvAttention: LayerNorm -> 1x1-conv QKV ->
per-(b,h)-row attention over W -> skip connection.

Sharding: data-parallel over batch B=8 across 8 NeuronCores. Each core
processes 64 (h) slabs of [W=256, C=256].

Numerics (gate: rel 2e-2; this lands ~4e-3):
- QKV projection: fp16 operands (xn^T via DMA-xbar transpose, W pre-cast on
  host). fp16 runs the PE at full rate (1.0 cycles/row); bf16 is NOT enough
  precision-wise: exp() turns absolute score error into relative softmax
  weight error, and bf16 operands give ~0.13 score error vs fp16's ~0.017.
- scores and y-matmuls: f32r operands (measured on HW: ~8e-3 max score
  error, better than fp16), also 1.0 cycles/row at out-free >= 256. f32r
  tiles are filled by plain bitwise DMA from PSUM (PSUM tiles are declared
  f32r), so no DVE/ACT cast op is spent on them at all.
- E = exp(s^T - 32) (constant shift; exact in real arithmetic), Z from two
  ones-columns appended to V inside the y-matmul.

Engine budget per slab (measured op costs): PE ~1.8us (14 matmuls),
DVE ~1.9us (bn_stats/aggr, fused (x-mu)*rs dual-scalar, reciprocal),
ACT ~1.1us (ln, exp rsqrt chain, exp E), GPSIMD ~1.4us (fused
(y*rZ)+x out), Sync ~1.3us (merged [128,512] DMA transpose). PSUM->SBUF
moves (qk, v, y) ride the 16 DMA queues.

Pipeline: 4-stage software pipeline (lag 1 per stage) so each engine's
in-order queue never head-of-line blocks on another engine's same-slab work.
"""

import os
import sys

for _p in ("/opt/trn_rl_repo", "/root/.axon_site/_ro/trn_rl_repo"):
    if _p not in sys.path:
        sys.path.insert(0, _p)

import numpy as np

import concourse.tile as tile
from concourse import bacc, mybir
from concourse.bass_utils import run_bass_kernel_spmd

F32 = mybir.dt.float32
F32R = mybir.dt.float32r
F16 = mybir.dt.float16
AF = mybir.ActivationFunctionType
ALU = mybir.AluOpType

B, H, W, C = 8, 64, 256, 256
F2 = 2 * C
NS = H  # slabs per core (batch-sharded over 8 cores)
EPS = 1e-3  # Keras LayerNormalization default
SHIFT = 32.0  # constant softmax shift (replaces per-row max subtraction)

_NC_CACHE: dict = {}


def _install_act_root():
    """Reorder act_info.json so natural_log_exp_and_others is the first set:
    bass' first-match table chooser then resolves Ln and Exp to that one set
    instead of alternating exp_and_others / natural_log every slab
    (129 table loads x ~2.7us)."""
    if os.environ.get("BASS_ACT_ROOT_JSON_PATH"):
        return
    try:
        import json
        import tempfile

        import neuronxcc.driver.jobs.support.FindActInfo as FAI
        from neuronxcc.driver.Job import Job

        src = FAI.findActInfoFile(Job.getPackageDir(), "gen3")
        srcdir = os.path.dirname(src)
        d = json.load(open(src))
        sets = d["act_func_sets"]
        first = [s for s in sets if s["name"] == "natural_log_exp_and_others"]
        if not first:
            return
        rest = [s for s in sets if s["name"] != "natural_log_exp_and_others"]
        d["act_func_sets"] = first + rest
        td = tempfile.mkdtemp(prefix="act_root_")
        for fn in os.listdir(srcdir):
            sp = os.path.join(srcdir, fn)
            if os.path.isfile(sp) and fn != os.path.basename(src):
                os.symlink(sp, os.path.join(td, fn))
        out = os.path.join(td, os.path.basename(src))
        with open(out, "w") as f:
            json.dump(d, f)
        os.environ["BASS_ACT_ROOT_JSON_PATH"] = out
        _orig = FAI.findActInfoFile
        FAI.findActInfoFile = lambda *a, **k: out
        import concourse.hw_specs as hw_specs

        hw_specs.get_activation_tables.cache_clear()
    except Exception as e:  # noqa: BLE001
        print(f"act root override failed (table thrash will persist): {e}")


def _build(with_bias: bool):
    _install_act_root()
    nc = bacc.Bacc("TRN2", target_bir_lowering=False, debug=False, num_devices=8)
    x_d = nc.dram_tensor("x", [NS, W, C], F32, kind="ExternalInput").ap()
    wqk_d = nc.dram_tensor("wqk", [2, 128, 256], F16, kind="ExternalInput").ap()
    wv_d = nc.dram_tensor("wv", [2, 128, 256], F16, kind="ExternalInput").ap()
    bqk_d = bv_d = None
    if with_bias:
        bqk_d = nc.dram_tensor("bqk", [2, 128], F32, kind="ExternalInput").ap()
        bv_d = nc.dram_tensor("bv", [256], F32, kind="ExternalInput").ap()
    out_d = nc.dram_tensor("out", [NS, W, C], F32, kind="ExternalOutput").ap()

    # per-slab views: [p=128, t(w-chunk)=2, c=256]
    x_r = x_d.rearrange("s (t p) c -> s p t c", p=128)
    out_r = out_d.rearrange("s (t p) c -> s p t c", p=128)

    with tile.TileContext(nc) as tc:
        _emit(nc, tc, x_r, out_r, wqk_d, wv_d, bqk_d, bv_d)
    nc.compile()
    return nc


def _emit(nc, tc, x_r, out_r, wqk_d, wv_d, bqk_d, bv_d):
    from contextlib import ExitStack

    with ExitStack() as ctx:
        ec = ctx.enter_context
        consts = ec(tc.tile_pool(name="consts", bufs=1))
        xpool = ec(tc.tile_pool(name="xp", bufs=9))
        xdpool = ec(tc.tile_pool(name="xdp", bufs=4))
        xnpool = ec(tc.tile_pool(name="xnp", bufs=4))
        xtpool = ec(tc.tile_pool(name="xtp", bufs=5))
        qkpool = ec(tc.tile_pool(name="qkp", bufs=4))
        epool = ec(tc.tile_pool(name="ep", bufs=4))
        vpool = ec(tc.tile_pool(name="vp", bufs=5))
        ypool = ec(tc.tile_pool(name="yp", bufs=2))
        opool = ec(tc.tile_pool(name="op", bufs=3))
        stat = ec(tc.tile_pool(name="stat", bufs=8))
        ps_qk = ec(tc.tile_pool(name="ps_qk", bufs=2, space="PSUM"))
        ps_sT = ec(tc.tile_pool(name="ps_sT", bufs=2, space="PSUM"))
        ps_v = ec(tc.tile_pool(name="ps_v", bufs=2, space="PSUM"))
        ps_y = ec(tc.tile_pool(name="ps_y", bufs=1, space="PSUM"))

        negshift = consts.tile([128, 1], F32)
        nc.vector.memset(negshift, -SHIFT)
        eps_t = consts.tile([128, 1], F32)
        nc.vector.memset(eps_t, EPS)

        wqk = consts.tile([128, 2, 256], F16)
        nc.sync.dma_start(wqk, wqk_d.rearrange("t p f -> p t f"))
        wv = consts.tile([128, 2, 256], F16)
        nc.sync.dma_start(wv, wv_d.rearrange("t p f -> p t f"))

        if bqk_d is not None:
            import concourse.bass as bass

            bqk_sb = consts.tile([128, 2], F32)
            nc.sync.dma_start(bqk_sb, bqk_d.rearrange("t p -> p t"))
            bvf = consts.tile([128, 2, 256], F32)
            bv_b = bass.AP(tensor=bv_d.tensor, offset=bv_d.offset,
                           ap=[[0, 128], [0, 2], [1, 256]])
            nc.sync.dma_start(bvf, bv_b)

        ring: dict = {}

        def stage_a(s):
            x_sb = xpool.tile([128, 2, 256], F32)
            nc.sync.dma_start(x_sb, x_r[s])
            # LayerNorm stats per row (partition = w position)
            st = stat.tile([128, 2, 6], F32)
            mv = stat.tile([128, 2, 2], F32)
            for t in (0, 1):
                nc.vector.bn_stats(st[:, t, :], x_sb[:, t, :])
                nc.vector.bn_aggr(mv[:, t, :], st[:, t, :])
            # rs = rsqrt(var + eps) = exp(-0.5 * ln(var + eps)); ln+exp share
            # one ACT table set (see _install_act_root)
            lnv = stat.tile([128, 2, 1], F32)
            nc.scalar.activation(out=lnv, in_=mv[:, :, 1:2], func=AF.Ln,
                                 bias=eps_t, scale=1.0)
            rs = stat.tile([128, 2, 1], F32)
            nc.scalar.activation(out=rs, in_=lnv, func=AF.Exp, scale=-0.5)
            # xn = (x - mu) * rs, straight to fp16. On gpsimd (the only
            # engine with slack; it cannot touch PSUM so all PSUM->SBUF
            # copies must stay on DVE/ACT). gpsimd tensor_TENSOR runs at
            # ~1.4ns/elem while its tensor_scalar form is a 15ns/elem slow
            # path -- so broadcast mu/rs via stride-0 APs instead.
            xd = xdpool.tile([128, 2, 256], F32)
            nc.gpsimd.tensor_tensor(
                out=xd, in0=x_sb,
                in1=mv[:, :, 0:1].broadcast_to([128, 2, 256]),
                op=ALU.subtract)
            xn = xnpool.tile([128, 2, 256], F16)
            nc.gpsimd.tensor_tensor(
                out=xn, in0=xd, in1=rs.broadcast_to([128, 2, 256]),
                op=ALU.mult)
            # one merged DMA-xbar transpose of [128, 512]: block q = t*2+cc
            # holds xn^T[cc*128:(cc+1)*128, t*128:(t+1)*128]
            xnT = xtpool.tile([128, 4, 128], F16)
            # issue on the ACT hwdge queue: on the Sync queue this transpose
            # (which waits for xn) head-of-line blocks the next slab's x load
            nc.scalar.dma_start(xnT, xn, transpose=True)
            ring[s] = {"x_sb": x_sb, "xnT": xnT, "rs": rs}

        def stage_b(s):
            r = ring[s]
            xnT = r["xnT"]
            xnT_c = xnT.rearrange("p (t c2) w -> p c2 t w", c2=2)
            # qk^T = Wqk^T @ xn^T : [f(2 blk), w]
            p_qk = ps_qk.tile([128, 2, 256], F32)
            for fb in (0, 1):
                for cc in (0, 1):
                    nc.tensor.matmul(
                        p_qk[:, fb, :],
                        wqk[:, cc, fb * 128:(fb + 1) * 128],
                        xnT_c[:, cc, :, :],
                        start=(cc == 0), stop=(cc == 1))
            if bqk_d is not None:
                for fb in (0, 1):
                    nc.vector.tensor_scalar(
                        out=p_qk[:, fb, :], in0=p_qk[:, fb, :],
                        scalar1=bqk_sb[:, fb:fb + 1], scalar2=None,
                        op0=ALU.add)
            # f32r operands run the scores matmul at full rate with ~2x
            # better precision than fp16 (measured on HW)
            qk_sb = qkpool.tile([128, 2, 256], F32R)
            nc.scalar.copy(qk_sb, p_qk)
            # v = xn @ Wv : [w(2 chunks), f]
            p_v = ps_v.tile([128, 2, 256], F32)
            for t in (0, 1):
                for cc in (0, 1):
                    nc.tensor.matmul(
                        p_v[:, t, :],
                        xnT[:, t * 2 + cc, :],
                        wv[:, cc, :],
                        start=(cc == 0), stop=(cc == 1))
            vb = vpool.tile([128, 2, 258], F32R)
            if bv_d is not None:
                nc.vector.tensor_tensor(out=vb[:, :, 0:256], in0=p_v,
                                        in1=bvf, op=ALU.add)
            else:
                nc.vector.tensor_copy(vb[:, :, 0:256], p_v)
            # ones columns: the y-matmul then also produces Z = sum_j E[j, i]
            nc.gpsimd.memset(vb[:, :, 256:258].bitcast(F32), 1.0)
            r["qk_sb"] = qk_sb
            r["vb"] = vb

        def stage_c(s):
            r = ring[s]
            qk_sb = r["qk_sb"]
            # s^T[j, i] = k @ q^T (contraction over d=128 on partitions)
            p_sT = ps_sT.tile([128, 2, 256], F32)
            for jt in (0, 1):
                nc.tensor.matmul(
                    p_sT[:, jt, :],
                    qk_sb[:, 1, jt * 128:(jt + 1) * 128],
                    qk_sb[:, 0, :],
                    start=True, stop=True)
            # E^T = exp(s^T - SHIFT), f32r for the y-matmul
            E = epool.tile([128, 2, 256], F32R)
            nc.scalar.activation(out=E, in_=p_sT, func=AF.Exp,
                                 bias=negshift, scale=1.0)
            r["E"] = E

        def stage_d(s):
            r = ring.pop(s)
            E, vb, x_sb = r["E"], r["vb"], r["x_sb"]
            # y[i, f] = E^T.T @ [v | 1]; cols 256/257 accumulate Z.
            # 512-wide it-chunks keep each matmul output inside one PSUM bank.
            p_y = ps_y.tile([128, 2, 512], F32)
            for it in (0, 1):
                for jt in (0, 1):
                    nc.tensor.matmul(
                        p_y[:, it, 0:258],
                        E[:, jt, it * 128:(it + 1) * 128],
                        vb[:, jt, :],
                        start=(jt == 0), stop=(jt == 1))
            rZ = stat.tile([128, 2, 1], F32)
            nc.vector.reciprocal(rZ, p_y[:, :, 256:257])
            # out = x + y * rZ; t=0 fused on DVE, t=1 split ACT-mul + gpsimd
            # add, so the PSUM reads spread across both PSUM-capable engines
            o_sb = opool.tile([128, 2, 256], F32)
            nc.vector.scalar_tensor_tensor(
                out=o_sb[:, 0, :], in0=p_y[:, 0, 0:256],
                scalar=rZ[:, 0, :], in1=x_sb[:, 0, :],
                op0=ALU.mult, op1=ALU.add)
            tmp1 = ypool.tile([128, 256], F32)
            nc.scalar.mul(tmp1, p_y[:, 1, 0:256], rZ[:, 1, :])
            nc.gpsimd.tensor_tensor(out=o_sb[:, 1, :], in0=tmp1,
                                    in1=x_sb[:, 1, :], op=ALU.add)
            nc.sync.dma_start(out_r[s], o_sb)

        LB, LC, LD = 3, 4, 5
        for i in range(NS + LD):
            if i < NS:
                stage_a(i)
            if 0 <= i - LB < NS:
                stage_b(i - LB)
            if 0 <= i - LC < NS:
                stage_c(i - LC)
            if 0 <= i - LD < NS:
                stage_d(i - LD)


def _install_ntff_hook():
    """Register the axon NTFF profiling hook (the image's antenv lacks
    axon_hooks, so boot skipped registration). Trace-only; best-effort."""
    try:
        import types

        import antenv

        if getattr(antenv, "axon_hooks", None) is not None:
            return
        mod = types.ModuleType("antenv.axon_hooks")
        _h = [None]
        mod.set_axon_ntff_profile_hook = lambda h: _h.__setitem__(0, h)
        mod.get_axon_ntff_profile_hook = lambda: _h[0]
        sys.modules["antenv.axon_hooks"] = mod
        antenv.axon_hooks = mod
        from trn_agent_boot.trn_boot import _ntff_profile_via_ctypes

        hook = _ntff_profile_via_ctypes("/opt/axon/libaxon_pjrt.so")
        if hook is not None:
            mod.set_axon_ntff_profile_hook(hook)
    except Exception as e:  # noqa: BLE001
        print(f"ntff hook install failed (timing unavailable): {e}")


def kernel(x, ln_gamma, ln_beta, W_qkv):
    x = np.asarray(x, dtype=np.float32)
    ln_gamma = np.asarray(ln_gamma, dtype=np.float32)
    ln_beta = np.asarray(ln_beta, dtype=np.float32)
    W_qkv = np.asarray(W_qkv, dtype=np.float32)
    assert x.shape == (B, H, W, C) and W_qkv.shape == (C, F2)

    # fold gamma/beta into the projection (1x1 conv has no bias of its own)
    Wp = (ln_gamma.astype(np.float64)[:, None] * W_qkv.astype(np.float64))
    bW = (ln_beta.astype(np.float64) @ W_qkv.astype(np.float64)).astype(np.float32)
    with_bias = bool(np.any(bW != 0.0))

    key = with_bias
    if key not in _NC_CACHE:
        _NC_CACHE[key] = _build(with_bias)
    nc = _NC_CACHE[key]

    wqk_f16 = Wp[:, :256].astype(np.float16).reshape(2, 128, 256)
    wv_f16 = Wp[:, 256:].astype(np.float16).reshape(2, 128, 256)
    in_maps = []
    for b in range(B):
        m = {
            "x": np.ascontiguousarray(x[b]),
            "wqk": np.ascontiguousarray(wqk_f16),
            "wv": np.ascontiguousarray(wv_f16),
        }
        if with_bias:
            m["bqk"] = np.ascontiguousarray(bW[:256].reshape(2, 128))
            m["bv"] = np.ascontiguousarray(bW[256:])
        in_maps.append(m)

    trace = os.environ.get("KERNEL_TRACE", "") == "1"
    if trace:
        _install_ntff_hook()
    res = run_bass_kernel_spmd(nc, in_maps, core_ids=list(range(B)), trace=trace)
    if trace and res.exec_time_ns is not None:
        print(f"HW exec time: {res.exec_time_ns} ns")
        if res.instructions_and_trace is not None:
            print(f"trace: {res.instructions_and_trace[1]}")
    out = np.stack([res.results[b]["out"] for b in range(B)], axis=0)
    return out.reshape(B, H, W, C).astype(np.float32, copy=False)


# revision 18
# speedup vs baseline: 1.0729x; 1.0729x over previous
"""Trainium2 Bass kernel for nn_ConvAttention: LayerNorm -> 1x1-conv QKV ->
per-(b,h)-row attention over W -> skip connection.

Sharding: data-parallel over batch B=8 across 8 NeuronCores. Each core
processes 64 (h) slabs of [W=256, C=256].

Numerics (gate: rel 2e-2; this lands ~4e-3):
- QKV projection: fp16 operands (xn^T via DMA-xbar transpose, W pre-cast on
  host). fp16 runs the PE at full rate (1.0 cycles/row); bf16 is NOT enough
  precision-wise: exp() turns absolute score error into relative softmax
  weight error, and bf16 operands give ~0.13 score error vs fp16's ~0.017.
- scores and y-matmuls: f32r operands (measured on HW: ~8e-3 max score
  error, better than fp16), also 1.0 cycles/row at out-free >= 256. f32r
  tiles are filled by plain bitwise DMA from PSUM (PSUM tiles are declared
  f32r), so no DVE/ACT cast op is spent on them at all.
- E = exp(s^T - 32) (constant shift; exact in real arithmetic), Z from two
  ones-columns appended to V inside the y-matmul.

Engine budget per slab (measured op costs): PE ~1.8us (14 matmuls),
DVE ~1.9us (bn_stats/aggr, fused (x-mu)*rs dual-scalar, reciprocal),
ACT ~1.1us (ln, exp rsqrt chain, exp E), GPSIMD ~1.4us (fused
(y*rZ)+x out), Sync ~1.3us (merged [128,512] DMA transpose). PSUM->SBUF
moves (qk, v, y) ride the 16 DMA queues.

Pipeline: 4-stage software pipeline (lag 1 per stage) so each engine's
in-order queue never head-of-line blocks on another engine's same-slab work.
"""

import os
import sys

for _p in ("/opt/trn_rl_repo", "/root/.axon_site/_ro/trn_rl_repo"):
    if _p not in sys.path:
        sys.path.insert(0, _p)

import numpy as np

import concourse.tile as tile
from concourse import bacc, mybir
from concourse.bass_utils import run_bass_kernel_spmd

F32 = mybir.dt.float32
F32R = mybir.dt.float32r
F16 = mybir.dt.float16
AF = mybir.ActivationFunctionType
ALU = mybir.AluOpType

B, H, W, C = 8, 64, 256, 256
F2 = 2 * C
NS = H  # slabs per core (batch-sharded over 8 cores)
EPS = 1e-3  # Keras LayerNormalization default
SHIFT = 32.0  # constant softmax shift (replaces per-row max subtraction)

_NC_CACHE: dict = {}


def _install_act_root():
    """Reorder act_info.json so natural_log_exp_and_others is the first set:
    bass' first-match table chooser then resolves Ln and Exp to that one set
    instead of alternating exp_and_others / natural_log every slab
    (129 table loads x ~2.7us)."""
    if os.environ.get("BASS_ACT_ROOT_JSON_PATH"):
        return
    try:
        import json
        import tempfile

        import neuronxcc.driver.jobs.support.FindActInfo as FAI
        from neuronxcc.driver.Job import Job

        src = FAI.findActInfoFile(Job.getPackageDir(), "gen3")
        srcdir = os.path.dirname(src)
        d = json.load(open(src))
        sets = d["act_func_sets"]
        first = [s for s in sets if s["name"] == "natural_log_exp_and_others"]
        if not first:
            return
        rest = [s for s in sets if s["name"] != "natural_log_exp_and_others"]
        d["act_func_sets"] = first + rest
        td = tempfile.mkdtemp(prefix="act_root_")
        for fn in os.listdir(srcdir):
            sp = os.path.join(srcdir, fn)
            if os.path.isfile(sp) and fn != os.path.basename(src):
                os.symlink(sp, os.path.join(td, fn))
        out = os.path.join(td, os.path.basename(src))
        with open(out, "w") as f:
            json.dump(d, f)
        os.environ["BASS_ACT_ROOT_JSON_PATH"] = out
        _orig = FAI.findActInfoFile
        FAI.findActInfoFile = lambda *a, **k: out
        import concourse.hw_specs as hw_specs

        hw_specs.get_activation_tables.cache_clear()
    except Exception as e:  # noqa: BLE001
        print(f"act root override failed (table thrash will persist): {e}")


def _build(with_bias: bool):
    _install_act_root()
    nc = bacc.Bacc("TRN2", target_bir_lowering=False, debug=False, num_devices=8)
    x_d = nc.dram_tensor("x", [NS, W, C], F32, kind="ExternalInput").ap()
    wqk_d = nc.dram_tensor("wqk", [2, 128, 256], F16, kind="ExternalInput").ap()
    wv_d = nc.dram_tensor("wv", [2, 128, 256], F16, kind="ExternalInput").ap()
    bqk_d = bv_d = None
    if with_bias:
        bqk_d = nc.dram_tensor("bqk", [2, 128], F32, kind="ExternalInput").ap()
        bv_d = nc.dram_tensor("bv", [256], F32, kind="ExternalInput").ap()
    out_d = nc.dram_tensor("out", [NS, W, C], F32, kind="ExternalOutput").ap()

    # per-slab views: [p=128, t(w-chunk)=2, c=256]
    x_r = x_d.rearrange("s (t p) c -> s p t c", p=128)
    out_r = out_d.rearrange("s (t p) c -> s p t c", p=128)

    with tile.TileContext(nc) as tc:
        _emit(nc, tc, x_r, out_r, wqk_d, wv_d, bqk_d, bv_d)
    nc.compile()
    return nc


def _emit(nc, tc, x_r, out_r, wqk_d, wv_d, bqk_d, bv_d):
    from contextlib import ExitStack

    with ExitStack() as ctx:
        ec = ctx.enter_context
        consts = ec(tc.tile_pool(name="consts", bufs=1))
        xpool = ec(tc.tile_pool(name="xp", bufs=9))
        xdpool = ec(tc.tile_pool(name="xdp", bufs=4))
        xnpool = ec(tc.tile_pool(name="xnp", bufs=4))
        xtpool = ec(tc.tile_pool(name="xtp", bufs=5))
        qkpool = ec(tc.tile_pool(name="qkp", bufs=4))
        epool = ec(tc.tile_pool(name="ep", bufs=4))
        vpool = ec(tc.tile_pool(name="vp", bufs=5))
        ypool = ec(tc.tile_pool(name="yp", bufs=2))
        opool = ec(tc.tile_pool(name="op", bufs=3))
        stat = ec(tc.tile_pool(name="stat", bufs=8))
        ps_qk = ec(tc.tile_pool(name="ps_qk", bufs=2, space="PSUM"))
        ps_sT = ec(tc.tile_pool(name="ps_sT", bufs=2, space="PSUM"))
        ps_v = ec(tc.tile_pool(name="ps_v", bufs=2, space="PSUM"))
        ps_y = ec(tc.tile_pool(name="ps_y", bufs=1, space="PSUM"))

        negshift = consts.tile([128, 1], F32)
        nc.vector.memset(negshift, -SHIFT)
        eps_t = consts.tile([128, 1], F32)
        nc.vector.memset(eps_t, EPS)

        wqk = consts.tile([128, 2, 256], F16)
        nc.sync.dma_start(wqk, wqk_d.rearrange("t p f -> p t f"))
        wv = consts.tile([128, 2, 256], F16)
        nc.sync.dma_start(wv, wv_d.rearrange("t p f -> p t f"))

        if bqk_d is not None:
            import concourse.bass as bass

            bqk_sb = consts.tile([128, 2], F32)
            nc.sync.dma_start(bqk_sb, bqk_d.rearrange("t p -> p t"))
            bvf = consts.tile([128, 2, 256], F32)
            bv_b = bass.AP(tensor=bv_d.tensor, offset=bv_d.offset,
                           ap=[[0, 128], [0, 2], [1, 256]])
            nc.sync.dma_start(bvf, bv_b)

        ring: dict = {}

        def stage_a(s):
            x_sb = xpool.tile([128, 2, 256], F32)
            nc.sync.dma_start(x_sb, x_r[s])
            # LayerNorm stats per row (partition = w position)
            st = stat.tile([128, 2, 6], F32)
            mv = stat.tile([128, 2, 2], F32)
            for t in (0, 1):
                nc.vector.bn_stats(st[:, t, :], x_sb[:, t, :])
                nc.vector.bn_aggr(mv[:, t, :], st[:, t, :])
            # rs = rsqrt(var + eps) = exp(-0.5 * ln(var + eps)); ln+exp share
            # one ACT table set (see _install_act_root)
            lnv = stat.tile([128, 2, 1], F32)
            nc.scalar.activation(out=lnv, in_=mv[:, :, 1:2], func=AF.Ln,
                                 bias=eps_t, scale=1.0)
            rs = stat.tile([128, 2, 1], F32)
            nc.scalar.activation(out=rs, in_=lnv, func=AF.Exp, scale=-0.5)
            # xn = (x - mu) * rs, straight to fp16. On gpsimd (the only
            # engine with slack; it cannot touch PSUM so all PSUM->SBUF
            # copies must stay on DVE/ACT). gpsimd tensor_TENSOR runs at
            # ~1.4ns/elem while its tensor_scalar form is a 15ns/elem slow
            # path -- so broadcast mu/rs via stride-0 APs instead.
            xd = xdpool.tile([128, 2, 256], F32)
            nc.gpsimd.tensor_tensor(
                out=xd, in0=x_sb,
                in1=mv[:, :, 0:1].broadcast_to([128, 2, 256]),
                op=ALU.subtract)
            xn = xnpool.tile([128, 2, 256], F16)
            nc.gpsimd.tensor_tensor(
                out=xn, in0=xd, in1=rs.broadcast_to([128, 2, 256]),
                op=ALU.mult)
            # one merged DMA-xbar transpose of [128, 512]: block q = t*2+cc
            # holds xn^T[cc*128:(cc+1)*128, t*128:(t+1)*128]
            ring[s] = {"x_sb": x_sb, "xn": xn}

        def stage_t(s):
            # transpose emitted one iteration after xn is produced, so it
            # never head-of-line blocks its queue waiting for same-iteration
            # gpsimd work. Block q = t*2+cc holds
            # xn^T[cc*128:(cc+1)*128, t*128:(t+1)*128].
            r = ring[s]
            xnT = xtpool.tile([128, 4, 128], F16)
            nc.sync.dma_start(xnT, r.pop("xn"), transpose=True)
            r["xnT"] = xnT

        def stage_b(s):
            r = ring[s]
            xnT = r["xnT"]
            xnT_c = xnT.rearrange("p (t c2) w -> p c2 t w", c2=2)
            # qk^T = Wqk^T @ xn^T : [f(2 blk), w]
            p_qk = ps_qk.tile([128, 2, 256], F32)
            for fb in (0, 1):
                for cc in (0, 1):
                    nc.tensor.matmul(
                        p_qk[:, fb, :],
                        wqk[:, cc, fb * 128:(fb + 1) * 128],
                        xnT_c[:, cc, :, :],
                        start=(cc == 0), stop=(cc == 1))
            if bqk_d is not None:
                for fb in (0, 1):
                    nc.vector.tensor_scalar(
                        out=p_qk[:, fb, :], in0=p_qk[:, fb, :],
                        scalar1=bqk_sb[:, fb:fb + 1], scalar2=None,
                        op0=ALU.add)
            # f32r operands run the scores matmul at full rate with ~2x
            # better precision than fp16 (measured on HW)
            qk_sb = qkpool.tile([128, 2, 256], F32R)
            nc.scalar.copy(qk_sb, p_qk)
            # v = xn @ Wv : [w(2 chunks), f]
            p_v = ps_v.tile([128, 2, 256], F32)
            for t in (0, 1):
                for cc in (0, 1):
                    nc.tensor.matmul(
                        p_v[:, t, :],
                        xnT[:, t * 2 + cc, :],
                        wv[:, cc, :],
                        start=(cc == 0), stop=(cc == 1))
            vb = vpool.tile([128, 2, 258], F32R)
            if bv_d is not None:
                nc.vector.tensor_tensor(out=vb[:, :, 0:256], in0=p_v,
                                        in1=bvf, op=ALU.add)
            else:
                nc.vector.tensor_copy(vb[:, :, 0:256], p_v)
            # ones columns: the y-matmul then also produces Z = sum_j E[j, i]
            nc.gpsimd.memset(vb[:, :, 256:258].bitcast(F32), 1.0)
            r["qk_sb"] = qk_sb
            r["vb"] = vb

        def stage_c(s):
            r = ring[s]
            qk_sb = r["qk_sb"]
            # s^T[j, i] = k @ q^T (contraction over d=128 on partitions)
            p_sT = ps_sT.tile([128, 2, 256], F32)
            for jt in (0, 1):
                nc.tensor.matmul(
                    p_sT[:, jt, :],
                    qk_sb[:, 1, jt * 128:(jt + 1) * 128],
                    qk_sb[:, 0, :],
                    start=True, stop=True)
            # E^T = exp(s^T - SHIFT), f32r for the y-matmul
            E = epool.tile([128, 2, 256], F32R)
            nc.scalar.activation(out=E, in_=p_sT, func=AF.Exp,
                                 bias=negshift, scale=1.0)
            r["E"] = E

        def stage_d(s):
            r = ring.pop(s)
            E, vb, x_sb = r["E"], r["vb"], r["x_sb"]
            # y[i, f] = E^T.T @ [v | 1]; cols 256/257 accumulate Z.
            # 512-wide it-chunks keep each matmul output inside one PSUM bank.
            p_y = ps_y.tile([128, 2, 512], F32)
            for it in (0, 1):
                for jt in (0, 1):
                    nc.tensor.matmul(
                        p_y[:, it, 0:258],
                        E[:, jt, it * 128:(it + 1) * 128],
                        vb[:, jt, :],
                        start=(jt == 0), stop=(jt == 1))
            rZ = stat.tile([128, 2, 1], F32)
            nc.vector.reciprocal(rZ, p_y[:, :, 256:257])
            # out = x + y * rZ; t=0 fused on DVE, t=1 split ACT-mul + gpsimd
            # add, so the PSUM reads spread across both PSUM-capable engines
            o_sb = opool.tile([128, 2, 256], F32)
            nc.vector.scalar_tensor_tensor(
                out=o_sb[:, 0, :], in0=p_y[:, 0, 0:256],
                scalar=rZ[:, 0, :], in1=x_sb[:, 0, :],
                op0=ALU.mult, op1=ALU.add)
            tmp1 = ypool.tile([128, 256], F32)
            nc.scalar.mul(tmp1, p_y[:, 1, 0:256], rZ[:, 1, :])
            nc.gpsimd.tensor_tensor(out=o_sb[:, 1, :], in0=tmp1,
                                    in1=x_sb[:, 1, :], op=ALU.add)
            nc.sync.dma_start(out_r[s], o_sb)

        LT, LB, LC, LD = 1, 3, 4, 5
        for i in range(NS + LD):
            if i < NS:
                stage_a(i)
            if 0 <= i - LT < NS:
                stage_t(i - LT)
            if 0 <= i - LB < NS:
                stage_b(i - LB)
            if 0 <= i - LC < NS:
                stage_c(i - LC)
            if 0 <= i - LD < NS:
                stage_d(i - LD)


def _install_ntff_hook():
    """Register the axon NTFF profiling hook (the image's antenv lacks
    axon_hooks, so boot skipped registration). Trace-only; best-effort."""
    try:
        import types

        import antenv

        if getattr(antenv, "axon_hooks", None) is not None:
            return
        mod = types.ModuleType("antenv.axon_hooks")
        _h = [None]
        mod.set_axon_ntff_profile_hook = lambda h: _h.__setitem__(0, h)
        mod.get_axon_ntff_profile_hook = lambda: _h[0]
        sys.modules["antenv.axon_hooks"] = mod
        antenv.axon_hooks = mod
        from trn_agent_boot.trn_boot import _ntff_profile_via_ctypes

        hook = _ntff_profile_via_ctypes("/opt/axon/libaxon_pjrt.so")
        if hook is not None:
            mod.set_axon_ntff_profile_hook(hook)
    except Exception as e:  # noqa: BLE001
        print(f"ntff hook install failed (timing unavailable): {e}")


def kernel(x, ln_gamma, ln_beta, W_qkv):
    x = np.asarray(x, dtype=np.float32)
    ln_gamma = np.asarray(ln_gamma, dtype=np.float32)
    ln_beta = np.asarray(ln_beta, dtype=np.float32)
    W_qkv = np.asarray(W_qkv, dtype=np.float32)
    assert x.shape == (B, H, W, C) and W_qkv.shape == (C, F2)

    # fold gamma/beta into the projection (1x1 conv has no bias of its own)
    Wp = (ln_gamma.astype(np.float64)[:, None] * W_qkv.astype(np.float64))
    bW = (ln_beta.astype(np.float64) @ W_qkv.astype(np.float64)).astype(np.float32)
    with_bias = bool(np.any(bW != 0.0))

    key = with_bias
    if key not in _NC_CACHE:
        _NC_CACHE[key] = _build(with_bias)
    nc = _NC_CACHE[key]

    wqk_f16 = Wp[:, :256].astype(np.float16).reshape(2, 128, 256)
    wv_f16 = Wp[:, 256:].astype(np.float16).reshape(2, 128, 256)
    in_maps = []
    for b in range(B):
        m = {
            "x": np.ascontiguousarray(x[b]),
            "wqk": np.ascontiguousarray(wqk_f16),
            "wv": np.ascontiguousarray(wv_f16),
        }
        if with_bias:
            m["bqk"] = np.ascontiguousarray(bW[:256].reshape(2, 128))
            m["bv"] = np.ascontiguousarray(bW[256:])
        in_maps.append(m)

    trace = os.environ.get("KERNEL_TRACE", "") == "1"
    if trace:
        _install_ntff_hook()
    res = run_bass_kernel_spmd(nc, in_maps, core_ids=list(range(B)), trace=trace)
    if trace and res.exec_time_ns is not None:
        print(f"HW exec time: {res.exec_time_ns} ns")
        if res.instructions_and_trace is not None:
            print(f"trace: {res.instructions_and_trace[1]}")
    out = np.stack([res.results[b]["out"] for b in range(B)], axis=0)
    return out.reshape(B, H, W, C).astype(np.float32, copy=False)
